# revision 3
# baseline (speedup 1.0000x reference)
"""Cubic B-spline basis expansion on Trainium2, SPMD across 8 NeuronCores.

Problem: xs [131072] f32, B [4,4] f32 (ascending-power coeffs), n=2048, q=3.
Output [131072, 2048] f32: each row i is zeros except 4 contiguous values at
columns first_i..first_i+3 where first_i = floor(xs[i]) (H=1, T0=0), and
value[k] = sum_p (frac + (q-k))^p * B[k,p].

Strategy (data-parallel, no cross-core comms), scatter-only:
  - shard xs / output rows across 8 cores (16384 rows each)
  - run_bass_kernel_spmd / run_bass_via_pjrt pre-zero ExternalOutput buffers
    (documented contract in bass2jax.py: "kernels that don't write every
    element rely on that"), so the kernel skips the 128 MiB/core zero-fill
    the previous version spent ~375 us on and only scatters the 4 nonzero
    values per row (16 B each).
  - DVE computes first_i (rounding-mode-robust floor), the flattened
    chunk-relative element index (kept < 2^24 because the DVE int32 add
    routes through the f32 ALU), and the 4 polynomial values per row via
    Horner on host-precomputed shift-expanded coefficients.
  - gpsimd issues J=128 SWDGE indirect DMAs, one per 128-row block: the HW
    semantics (measured) are one descriptor per partition per call, address
    = the partition's first offset element, payload = the partition's
    contiguous source extent (16 B here). Multi-index-per-partition offset
    APs do NOT work on HW (CoreSim models them, HW ignores all but the
    first index), and 4 SWDGE queues gen no faster than 1 (measured).

Measured on HW (in-NEFF iteration slope, 8 cores): ~148 us/iteration
vs 412 us for the zero-fill+scatter baseline; rel err 5.4e-06.
Per-call cost ~1.16 us = SWDGE fixed overhead (994 ns) + 128 descriptors.
"""
import sys

import numpy as np

for _p in ("/opt/trn_rl_repo",):
    if _p not in sys.path:
        sys.path.insert(0, _p)

import concourse.bass as bass
import concourse.mybir as mybir
from concourse.bass_utils import run_bass_kernel_spmd

NS = 131072           # total samples
N = 2048              # knots (output columns)
Q = 3                 # spline order
NCORES = 8
R = NS // NCORES      # 16384 rows per core
P = 128               # SBUF partitions
J = R // P            # 128 columns: row = j*128 + p
# HW SWDGE indirect semantics (measured): one descriptor per partition per
# call, address = the partition's FIRST offset element, payload = the
# partition's whole contiguous source extent. So a call can scatter exactly
# 128 rows x 4 elements -> MCHUNK must be 1 (J=128 calls per iteration).
MCHUNK = 1

F32 = mybir.dt.float32
I32 = mybir.dt.int32
ALU = mybir.AluOpType


def _shifted_coeffs(B_np: np.ndarray) -> np.ndarray:
    """c[k, p]: coefficients of t^p in sum_p B[k,p] * (t + (Q-k))^p."""
    from numpy.polynomial import Polynomial
    Bc = np.asarray(B_np, dtype=np.float64)
    c = np.zeros((Q + 1, Q + 1), dtype=np.float64)
    for k in range(Q + 1):
        poly = Polynomial(Bc[k])(Polynomial([float(Q - k), 1.0]))
        cc = poly.coef
        c[k, :len(cc)] = cc
    return c


def _build(B_np: np.ndarray, iters: int = 1, mchunk: int = MCHUNK) -> bass.Bass:
    assert J % mchunk == 0 and mchunk <= 64
    assert mchunk == 1, "HW SWDGE uses one index per partition per call"
    nchunks = J // mchunk
    nc = bass.Bass("TRN2")
    xs_d = nc.dram_tensor("xs", [P, J], F32, kind="ExternalInput")
    ib_d = nc.dram_tensor("ibase", [P, J], I32, kind="ExternalInput")
    out_d = nc.dram_tensor("out", [R, N], F32, kind="ExternalOutput")

    C = _shifted_coeffs(B_np)

    with (
        nc.sbuf_tensor("xs_t", [P, J], F32) as xs_t,
        nc.sbuf_tensor("ib_t", [P, J], I32) as ib_t,
        nc.sbuf_tensor("fi_f", [P, J], F32) as fi_f,
        nc.sbuf_tensor("gt_t", [P, J], F32) as gt_t,
        nc.sbuf_tensor("frac", [P, J], F32) as frac,
        nc.sbuf_tensor("fr2", [P, J], F32) as fr2,
        nc.sbuf_tensor("fi_i", [P, J], I32) as fi_i,
        nc.sbuf_tensor("idx", [P, J], I32) as idx,
        nc.sbuf_tensor("bk", [P, (Q + 1) * J], F32) as bk,
        nc.sbuf_tensor("vals", [P, (Q + 1) * J], F32) as vals,
        nc.semaphore("xsem") as xsem,
        nc.semaphore("csem") as csem,
        nc.semaphore("ssem") as ssem,
        nc.semaphore("vsem") as vsem,
    ):
        with nc.Block() as block:

            @block.vector
            def _(v):
                nv = 0

                def step(inst):
                    nonlocal nv
                    inst.then_inc(vsem, 1)
                    nv += 1

                def fence():
                    v.wait_ge(vsem, nv)

                v.wait_ge(xsem, 32)
                # first_i = floor(xs) for xs >= 0, robust to any f32->i32
                # rounding mode
                step(v.tensor_copy(out=fi_i[:], in_=xs_t[:]))
                fence()
                step(v.tensor_copy(out=fi_f[:], in_=fi_i[:]))
                fence()
                step(v.tensor_tensor(out=gt_t[:], in0=fi_f[:], in1=xs_t[:],
                                     op=ALU.is_gt))
                fence()
                step(v.tensor_tensor(out=fi_f[:], in0=fi_f[:], in1=gt_t[:],
                                     op=ALU.subtract))
                fence()
                step(v.tensor_tensor(out=frac[:], in0=xs_t[:], in1=fi_f[:],
                                     op=ALU.subtract))
                step(v.tensor_copy(out=fi_i[:], in_=fi_f[:]))
                fence()
                # idx = ((j % mchunk)*128 + p)*N + first_i  (< 2^24 so the
                # DVE f32 ALU int-add path is exact)
                step(v.tensor_tensor(out=idx[:], in0=ib_t[:], in1=fi_i[:],
                                     op=ALU.add))
                step(v.tensor_tensor(out=fr2[:], in0=frac[:], in1=frac[:],
                                     op=ALU.mult))
                # value_k = (c0 + c1*f) + f^2*(c2 + c3*f)
                vv = vals[:].rearrange("p (j k) -> p j k", k=Q + 1)
                bb = bk[:].rearrange("p (j k) -> p j k", k=Q + 1)
                for k in range(Q + 1):
                    c0, c1, c2, c3 = (float(C[k, 0]), float(C[k, 1]),
                                      float(C[k, 2]), float(C[k, 3]))
                    step(v.tensor_scalar(out=vv[:, :, k], in0=frac[:],
                                         scalar1=c1, scalar2=c0,
                                         op0=ALU.mult, op1=ALU.add))
                    step(v.tensor_scalar(out=bb[:, :, k], in0=frac[:],
                                         scalar1=c3, scalar2=c2,
                                         op0=ALU.mult, op1=ALU.add))
                fence()
                for k in range(Q + 1):
                    step(v.tensor_tensor(out=bb[:, :, k], in0=bb[:, :, k],
                                         in1=fr2[:], op=ALU.mult))
                fence()
                for k in range(Q + 1):
                    step(v.tensor_tensor(out=vv[:, :, k], in0=vv[:, :, k],
                                         in1=bb[:, :, k], op=ALU.add))
                fence()
                v.sem_inc(csem, 1)

            @block.gpsimd
            def _(g):
                g.dma_start(out=xs_t[:], in_=xs_d[:]).then_inc(xsem, 16)
                g.dma_start(out=ib_t[:], in_=ib_d[:]).then_inc(xsem, 16)
                g.wait_ge(csem, 1)
                for _it in range(iters):
                    for ci in range(nchunks):
                        j0 = ci * mchunk
                        g.indirect_dma_start(
                            out=out_d[:],
                            out_offset=bass.IndirectOffsetOnAxis(
                                ap=idx[:, j0:j0 + mchunk], axis=1),
                            in_=vals[:, (Q + 1) * j0:(Q + 1) * (j0 + mchunk)],
                            in_offset=None,
                            element_offset=j0 * P * N,
                        ).then_inc(ssem, 16)
                g.wait_ge(ssem, 16 * nchunks * iters)

    return nc


_CACHE: dict[tuple, bass.Bass] = {}


def _get_program(B: np.ndarray, iters: int = 1) -> bass.Bass:
    key = (np.asarray(B, dtype=np.float32).tobytes(), iters)
    if key not in _CACHE:
        _CACHE[key] = _build(B, iters=iters)
    return _CACHE[key]


def _in_maps(xs: np.ndarray) -> list[dict[str, np.ndarray]]:
    # j-major row layout: xs2d[p, j] = xs_shard[j*P + p];
    # ibase[p, j] = ((j % MCHUNK)*P + p) * N, chunk-local flattened base
    ibase = (((np.arange(J, dtype=np.int64)[None, :] % MCHUNK) * P
              + np.arange(P, dtype=np.int64)[:, None]) * N).astype(np.int32)
    maps = []
    for c in range(NCORES):
        shard = np.asarray(xs[c * R:(c + 1) * R], dtype=np.float32)
        xs2d = np.ascontiguousarray(shard.reshape(J, P).T)
        maps.append({"xs": xs2d, "ibase": ibase})
    return maps


def kernel(xs, B, n, q):
    xs = np.asarray(xs, dtype=np.float32)
    B = np.asarray(B, dtype=np.float32)
    n = int(np.asarray(n)) if not isinstance(n, int) else n
    q = int(np.asarray(q)) if not isinstance(q, int) else q
    assert xs.shape == (NS,), xs.shape
    assert B.shape == (Q + 1, Q + 1), B.shape
    assert n == N and q == Q, (n, q)

    nc = _get_program(B)
    try:
        res = run_bass_kernel_spmd(nc, _in_maps(xs), core_ids=list(range(NCORES)))
    except Exception:
        res = run_bass_kernel_spmd(nc, _in_maps(xs), core_ids=list(range(NCORES)))
    return np.concatenate([res.results[c]["out"] for c in range(NCORES)], axis=0)
